# revision 9
# baseline (speedup 1.0000x reference)
"""AtomicCharge Trainium2 kernel (nn_AtomicCharge_77781857730661).

Strategy
--------
Data-parallel over atoms across 8 NeuronCores. The host packs molecules
(contiguous runs of the sorted `batch` tensor) into 1024 partition-rows
(8 cores x 128 partitions) of capacity T=2048 slots, so every molecule
lives contiguously along the free dim of one partition. x is uploaded
pre-transposed (feature-major) per core.

On each core:
  1. Tiny MLP: h^T = silu(W1^T @ x^T + b1), atom_out = W2^T h + b2 via
     TensorE matmuls in float32r (full-rate fp32 path). mm2 uses per-pair
     "placement" stationaries so 512-atom chunk results accumulate directly
     into the packed [128, 2048] layout in PSUM (one panel per 512-block).
  2. Segment reductions as dense ops: per-molecule sums via two segmented
     scans (forward + reversed via negative-stride APs) with host-built
     0/1 reset masks; per-molecule charge and 1/count expanded by
     segmented hold-scans of host-placed values.
  3. out = atom_out + (charge[b] - sum[b]) * (1/cnt[b]) elementwise.

Host reassembles the 8 output grids and gathers per-atom values back.
"""
import sys

sys.path.insert(0, "/opt/trn_rl_repo")

import numpy as np

import concourse.bass as bass
from concourse import mybir
from concourse.bass_utils import run_bass_kernel_spmd

F32 = mybir.dt.float32
F32R = mybir.dt.float32r

# problem constants (hardcoded per spec)
N_ATOMS = 2_000_000
N_MOL = 50_000
D = 128      # node feature dim = SBUF partitions
H = 64       # hidden dim
NCORES = 8
R = 128      # atom-layout rows per core (partitions)
T = 2048     # slots per row
NB = 512     # free block size per matmul chunk
S = R * T    # padded atoms per core
NPAIR = R // 2
NBLK = T // NB

_NC_CACHE = {}
LAST_RUN_INFO = {}


def build_raw(D=128, H=64, R=128, T=2048, NB=512, use_silu=True):
    NPAIR = R // 2
    NBLK = T // NB
    NCH = NPAIR * NBLK          # total 512-atom chunks
    S = R * T
    AOp = mybir.AluOpType

    nc = bass.Bass()
    xT = nc.declare_dram_parameter("xT", [D, S], F32R, isOutput=False)
    W1 = nc.declare_dram_parameter("W1", [D, H], F32R, isOutput=False)
    b1s = nc.declare_dram_parameter("b1s", [D], F32, isOutput=False)
    b2 = nc.declare_dram_parameter("b2", [1], F32, isOutput=False)
    W2p = nc.declare_dram_parameter("W2p", [D, NPAIR * D], F32R, isOutput=False)
    aux = nc.declare_dram_parameter("aux", [R, 4 * T], F32, isOutput=False)
    out = nc.declare_dram_parameter("out", [R, T], F32, isOutput=True)

    from contextlib import ExitStack
    with ExitStack() as ctx:
        def sbuf(shape, dtype, name):
            return ctx.enter_context(nc.sbuf_tensor(name, shape, dtype))

        def psum(shape, name):
            return ctx.enter_context(nc.psum_tensor(name, shape, F32))

        w1 = sbuf([D, H], F32R, "w1")
        b1t = sbuf([D, 1], F32, "b1t")
        b2t = sbuf([D, 1], F32, "b2t")
        w2place = sbuf([D, NPAIR * D], F32R, "w2place")
        auxt = sbuf([R, 4 * T], F32, "auxt")
        xp = [sbuf([D, 2 * T], F32R, f"xp{s}") for s in range(2)]
        hs = [sbuf([D, NB], F32R, f"hs{s}") for s in range(3)]
        ao = sbuf([R, T], F32, "ao")
        FL = sbuf([R, T], F32, "FL")
        RLr = sbuf([R, T], F32, "RLr")
        CH = sbuf([R, T], F32, "CH")
        IV = sbuf([R, T], F32, "IV")

        hpA = [psum([H, NB], f"hpA{s}") for s in range(2)]
        hpB = [psum([H, NB], f"hpB{s}") for s in range(2)]
        panels = [psum([R, NB], f"panel{j}") for j in range(NBLK)]

        s_aux = ctx.enter_context(nc.semaphore("s_aux"))
        s_x = [ctx.enter_context(nc.semaphore(f"s_x{i}")) for i in range(2)]
        s_hp = ctx.enter_context(nc.semaphore("s_hp"))
        s_hs = ctx.enter_context(nc.semaphore("s_hs"))
        s_mm2 = ctx.enter_context(nc.semaphore("s_mm2"))
        s_dve = ctx.enter_context(nc.semaphore("s_dve"))
        s_epi = ctx.enter_context(nc.semaphore("s_epi"))
        s_out = ctx.enter_context(nc.semaphore("s_out"))
        block = ctx.enter_context(nc.Block())

        mAt = auxt[:, 0 * T:1 * T]
        mBrt = auxt[:, 1 * T:2 * T]
        pCHt = auxt[:, 2 * T:3 * T]
        pIVt = auxt[:, 3 * T:4 * T]

        def rev(ap):
            return bass.AP(tensor=ap.tensor, offset=ap.offset + (T - 1),
                           ap=[list(ap.ap[0]), [-1, T]])

        # ---------------- SP: all DMA traffic ----------------
        @block.sync
        def _(sync):
            sync.dma_start(out=w1[:], in_=W1[:]).then_inc(s_aux, 16)
            sync.dma_start(out=b1t[:], in_=b1s[:, None]).then_inc(s_aux, 16)
            b2bc = bass.AP(tensor=b2.ap().tensor, offset=0, ap=[[0, D], [1, 1]])
            sync.dma_start(out=b2t[:], in_=b2bc).then_inc(s_aux, 16)
            sync.dma_start(out=w2place[:], in_=W2p[:]).then_inc(s_aux, 16)
            sync.dma_start(out=auxt[:], in_=aux[:]).then_inc(s_aux, 16)
            for k in range(NPAIR):
                if k >= 2:
                    # xp slot free once pair k-2's mm1s done
                    sync.wait_ge(s_hp, 2 * NBLK * (k - 1))
                sync.dma_start(out=xp[k % 2][:],
                               in_=xT[:, 2 * k * T:(2 * k + 2) * T]
                               ).then_inc(s_x[k % 2], 16)
            sync.wait_ge(s_epi, 1)
            sync.dma_start(out=out[:], in_=ao[:]).then_inc(s_out, 16)
            sync.wait_ge(s_out, 16)

        # ---------------- PE: matmul pipeline ----------------
        @block.tensor
        def _(tensor):
            tensor.wait_ge(s_aux, 80)
            for i in range(NCH):
                k, j = divmod(i, NBLK)
                if j == 0:
                    tensor.wait_ge(s_x[k % 2], 16 * (k // 2 + 1))
                if i >= 1:
                    tensor.wait_ge(s_hs, 2 * i)
                    k2, j2 = divmod(i - 1, NBLK)
                    nc.tensor.matmul(
                        out=panels[j2][:],
                        lhsT=w2place[:, k2 * D:(k2 + 1) * D],
                        rhs=hs[(i - 1) % 3][:],
                        start=(k2 == 0), stop=(k2 == NPAIR - 1),
                    ).then_inc(s_mm2, 1)
                xslot = xp[k % 2]
                nc.tensor.matmul(out=hpA[i % 2][:], lhsT=w1[:],
                                 rhs=xslot[:, j * NB:(j + 1) * NB],
                                 start=True, stop=True).then_inc(s_hp, 1)
                nc.tensor.matmul(out=hpB[i % 2][:], lhsT=w1[:],
                                 rhs=xslot[:, T + j * NB:T + (j + 1) * NB],
                                 start=True, stop=True).then_inc(s_hp, 1)
            tensor.wait_ge(s_hs, 2 * NCH)
            k2, j2 = divmod(NCH - 1, NBLK)
            nc.tensor.matmul(
                out=panels[j2][:],
                lhsT=w2place[:, k2 * D:(k2 + 1) * D],
                rhs=hs[(NCH - 1) % 3][:],
                start=(k2 == 0), stop=(k2 == NPAIR - 1),
            ).then_inc(s_mm2, 1)

        # ---------------- ACT: silu ----------------
        @block.scalar
        def _(scalar):
            func = (mybir.ActivationFunctionType.Silu if use_silu
                    else mybir.ActivationFunctionType.Sigmoid)
            scalar.wait_ge(s_aux, 80)
            for i in range(NCH):
                scalar.wait_ge(s_hp, 2 * (i + 1))
                if i >= 3:
                    scalar.wait_ge(s_mm2, i - 2)
                nc.scalar.activation(
                    out=hs[i % 3][0:H, :], in_=hpA[i % 2][:],
                    func=func, bias=b1t[0:H], scale=1.0,
                ).then_inc(s_hs, 1)
                nc.scalar.activation(
                    out=hs[i % 3][H:D, :], in_=hpB[i % 2][:],
                    func=func, bias=b1t[0:H], scale=1.0,
                ).then_inc(s_hs, 1)

        # ---------------- DVE: w2place build, segment phase ----------------
        @block.vector
        def _(vector):
            tick = [0]

            def step(ins):
                # serialize the DVE chain with explicit sems (deep-pipeline
                # same-engine hazards are real; race detector enforces)
                ins.then_inc(s_dve, 1)
                tick[0] += 1
                vector.wait_ge(s_dve, tick[0])

            vector.wait_ge(s_aux, 80)
            vector.wait_ge(s_mm2, NCH)
            for j in range(NBLK):
                step(nc.vector.tensor_scalar_add(
                    ao[:, j * NB:(j + 1) * NB], panels[j][:], b2t[:]))
            step(nc.vector.tensor_tensor_scan(
                out=FL[:], data0=mAt, data1=ao[:],
                initial=0.0, op0=AOp.mult, op1=AOp.add))
            step(nc.vector.tensor_tensor_scan(
                out=RLr[:], data0=mBrt, data1=rev(ao[:]),
                initial=0.0, op0=AOp.mult, op1=AOp.add))
            step(nc.vector.tensor_tensor_scan(
                out=CH[:], data0=mAt, data1=pCHt,
                initial=0.0, op0=AOp.mult, op1=AOp.add))
            step(nc.vector.tensor_tensor_scan(
                out=IV[:], data0=mAt, data1=pIVt,
                initial=0.0, op0=AOp.mult, op1=AOp.add))
            u = mAt      # aux slices are free now
            step(nc.vector.tensor_add(u, FL[:], rev(RLr[:])))
            step(nc.vector.tensor_sub(u, u, ao[:]))
            step(nc.vector.tensor_sub(u, CH[:], u))
            step(nc.vector.tensor_mul(u, u, IV[:]))
            nc.vector.tensor_add(ao[:], ao[:], u).then_inc(s_epi, 1)

    return nc


def build_nc(use_silu=True):
    key = (use_silu,)
    if key in _NC_CACHE:
        return _NC_CACHE[key]
    nc = build_raw(D=D, H=H, R=R, T=T, NB=NB, use_silu=use_silu)
    _NC_CACHE[key] = nc
    return nc


def _pack(batch, charge):
    """Pack molecules into 1024 rows of capacity T. Returns per-atom slot
    positions and the per-core host-side input grids."""
    n = batch.shape[0]
    sizes = np.bincount(batch, minlength=N_MOL).astype(np.int64)
    nz = np.flatnonzero(sizes)           # non-empty molecules, in order
    szs = sizes[nz]
    nrows = NCORES * R

    # greedy sequential packing of molecules into rows
    row_of = np.empty(len(nz), np.int64)
    fstart = np.empty(len(nz), np.int64)
    r, f = 0, 0
    for i, sz in enumerate(szs):
        if f + sz > T:
            r += 1
            f = 0
        row_of[i] = r
        fstart[i] = f
        f += sz
    assert r < nrows, f"packing overflowed: needed {r + 1} rows > {nrows}"

    slot_start = row_of * T + fstart     # global slot of each molecule start
    # per-atom global slot: atoms of molecule i occupy slot_start[i] + 0..sz
    mol_atom_start = np.concatenate([[0], np.cumsum(szs)])[:-1]
    # batch is sorted, so atom a belongs to the idx-th non-empty molecule
    idx_of_atom = np.repeat(np.arange(len(nz)), szs)
    pos_of_atom = slot_start[idx_of_atom] + (np.arange(n) - mol_atom_start[idx_of_atom])

    # masks / placed values over all rows
    fill = np.zeros(nrows, np.int64)
    np.add.at(fill, row_of, szs)
    col = np.arange(T)
    mA = np.ones((nrows, T), np.float32)
    mA.reshape(-1)[slot_start] = 0.0
    mA[col[None, :] >= fill[:, None]] = 0.0
    slot_end = slot_start + szs - 1
    mBr = np.ones((nrows, T), np.float32)
    # reversed coords: slot (r, f) -> (r, T-1-f)
    mBr.reshape(-1)[(slot_end // T) * T + (T - 1 - (slot_end % T))] = 0.0
    # pad slots in reversed coords are cols < T - fill
    mBr[col[None, :] < (T - fill[:, None])] = 0.0

    pCH = np.zeros((nrows, T), np.float32)
    pCH.reshape(-1)[slot_start] = charge[nz]
    pIV = np.zeros((nrows, T), np.float32)
    pIV.reshape(-1)[slot_start] = (1.0 / szs).astype(np.float32)

    return pos_of_atom, mA, mBr, pCH, pIV


def _round_f32r(a):
    """Round fp32 array to fp32r (tf32-like: low 12 mantissa bits zero), RNE."""
    v = np.ascontiguousarray(a, dtype=np.float32).view(np.uint32)
    r = (v + 0x7FF + ((v >> 12) & 1)) & np.uint32(0xFFFFF000)
    return r.view(np.float32)


def kernel(x_scalar, batch, charge, W1, b1, W2, b2):
    x_scalar = np.asarray(x_scalar, dtype=np.float32)
    batch = np.asarray(batch, dtype=np.int32)
    charge = np.asarray(charge, dtype=np.float32)
    W1 = np.asarray(W1, dtype=np.float32)
    b1 = np.asarray(b1, dtype=np.float32)
    W2 = np.asarray(W2, dtype=np.float32)
    b2 = np.asarray(b2, dtype=np.float32)
    n = x_scalar.shape[0]

    # tolerate unsorted batch (reference data is sorted; this is insurance)
    order = None
    if np.any(np.diff(batch) < 0):
        order = np.argsort(batch, kind="stable")
        x_scalar = x_scalar[order]
        batch = batch[order]

    pos_of_atom, mA, mBr, pCH, pIV = _pack(batch, charge)

    # padded, packed, transposed x per core
    xpad = np.zeros((NCORES * S, D), np.float32)
    xpad[pos_of_atom] = _round_f32r(x_scalar)
    xT_cores = [np.ascontiguousarray(xpad[c * S:(c + 1) * S].T)
                for c in range(NCORES)]
    del xpad

    W2p = np.zeros((D, NPAIR * D), np.float32)
    for k in range(NPAIR):
        W2p[:H, k * D + 2 * k] = W2[:, 0]
        W2p[H:, k * D + 2 * k + 1] = W2[:, 0]
    W2p = _round_f32r(W2p)
    W1 = _round_f32r(W1)
    b1s = np.concatenate([b1, b1]).astype(np.float32)

    nc = build_nc(use_silu=True)
    in_maps = []
    for c in range(NCORES):
        sl = slice(c * R, (c + 1) * R)
        auxc = np.concatenate([mA[sl], mBr[sl], pCH[sl], pIV[sl]], axis=1)
        in_maps.append({
            "xT": xT_cores[c], "W1": W1, "b1s": b1s, "b2": b2, "W2p": W2p,
            "aux": np.ascontiguousarray(auxc),
        })

    import os
    trace = bool(int(os.environ.get("ATOMIC_TRACE", "0")))
    res = run_bass_kernel_spmd(nc, in_maps, list(range(NCORES)), trace=trace)
    LAST_RUN_INFO["exec_time_ns"] = getattr(res, "exec_time_ns", None)
    LAST_RUN_INFO["profile_json"] = getattr(res, "profile_json", None)

    big = np.concatenate([res.results[c]["out"].reshape(-1)
                          for c in range(NCORES)])
    at = big[pos_of_atom].astype(np.float32)
    if order is not None:
        inv = np.empty_like(order)
        inv[order] = np.arange(n)
        at = at[inv]
    return at
